# revision 26
# baseline (speedup 1.0000x reference)
"""Causal multi-head attention (B=2, L=2048, D=2048, NH=16, HD=128) on 8
Trainium2 NeuronCores.

Sharding: core c = b*4 + g handles batch b and head-group g (4 heads).
Each core computes q/k/v projections for its 512 features, causal
attention for its 4 heads, and the partial o-projection
attn_out @ Wo[:, g_cols].T -> [L, D].  The host sums the 4 per-batch
partials and adds bo.

All matmul operands are bf16 (PSUM accumulation stays fp32): same PE
rate as full-rate f32r but half the HBM/SBUF traffic.  x^T is loaded
once into SBUF (64 KiB/partition) and reused for the q, k and v
projections; o partials are written back as bf16.

Structure:
  phase 1: q,k projections (wq+wk resident, x streamed in chunks into
           the resident x tile), then v projection for the whole
           sequence.  wo prefetched during this phase.
  phase 2: flash-style causal attention (scores in the transposed
           [k, q] layout; softmax without max-shift -- scores ~ N(0,1);
           row sums via ones-matmul; normalization by broadcast
           reciprocal) fused with the partial o-projection.
"""

import sys

for _p in ("/opt/trn_rl_repo",):
    if _p not in sys.path:
        sys.path.insert(0, _p)

import numpy as np
from contextlib import ExitStack

import concourse.bass as bass  # noqa: F401
import concourse.tile as tile
from concourse import bacc, mybir
from concourse import bass_utils

P = 128
B, L, D = 2, 2048, 2048
NH, HD = 16, 128
SCALE = HD ** -0.5
G = 8 // B            # head-groups per batch = 4
H = 4                 # heads per core
F = H * HD            # 512 features per core
TB = 512              # token block (q-block)
NTB = L // TB         # 4
KT = D // P           # 16 contraction tiles for projections

bf = mybir.dt.bfloat16
f32r = mybir.dt.float32r
f32 = mybir.dt.float32

_CACHE = {}


def _build(reps=1, phases="12", cfg=None):
    cfg = {**dict(stag=3, pts=4, psS=3, psPO=2, psR=1, psC=2), **(cfg or {})}
    key = ("nc", reps, phases, tuple(sorted(cfg.items())))
    if key in _CACHE:
        return _CACHE[key]

    nc = bacc.Bacc("TRN2", target_bir_lowering=False, debug=False, num_devices=8)

    xT = nc.dram_tensor("xT", [D, L], bf, kind="ExternalInput").ap()
    wq = nc.dram_tensor("wq", [P, H, KT, HD], bf, kind="ExternalInput").ap()
    wk = nc.dram_tensor("wk", [P, H, KT, HD], bf, kind="ExternalInput").ap()
    wv = nc.dram_tensor("wv", [P, KT, F], bf, kind="ExternalInput").ap()
    wo = nc.dram_tensor("wo", [P, H, D], bf, kind="ExternalInput").ap()
    bqv = nc.dram_tensor("bqv", [F], f32r, kind="ExternalInput").ap()
    bkv = nc.dram_tensor("bkv", [F], f32r, kind="ExternalInput").ap()
    o = nc.dram_tensor("o", [L, D], bf, kind="ExternalOutput").ap()

    xT3 = xT.rearrange("(kt p) t -> p kt t", p=P)

    with tile.TileContext(nc) as tc:
        with ExitStack() as ctx:
            ctx.enter_context(nc.allow_low_precision(reason="bf16 kernel"))
            consts = ctx.enter_context(tc.tile_pool(name="consts", bufs=1))
            resid = ctx.enter_context(tc.tile_pool(name="resid", bufs=1))

            # ---- constants ----
            # triangular additive mask [P, P] in [k, q] orientation:
            # keep (0.0) where k_local <= q_local, else -1e30
            tri = consts.tile([P, P], f32, name="tri")
            nc.gpsimd.memset(tri[:], 0.0)
            nc.gpsimd.affine_select(
                out=tri[:],
                in_=tri[:],
                compare_op=mybir.AluOpType.is_ge,
                fill=-1e30,
                base=0,
                pattern=[[1, P]],
                channel_multiplier=-1,
            )

            # all-ones stationary for the rsum colsum matmul; full 128x128
            # keeps the PE weight-load geometry uniform with every other
            # stationary, and makes the rsum output row-broadcast
            ones_sq = consts.tile([P, P], bf, name="ones_sq")
            nc.gpsimd.memset(ones_sq[:], 1.0)
            # per-partition bias tiles for the q/k copies: [P, H]
            bq_pp = consts.tile([P, H], f32, name="bq_pp")
            nc.scalar.dma_start(bq_pp[:], bqv.rearrange("(h p) -> p h", p=P).bitcast(f32))
            bk_pp = consts.tile([P, H], f32, name="bk_pp")
            nc.scalar.dma_start(bk_pp[:], bkv.rearrange("(h p) -> p h", p=P).bitcast(f32))

            # ---- persistent activations ----
            xr = resid.tile([P, KT, L], bf, name="xr")            # 64 KiB/part
            qT_sb = resid.tile([P, H, L], bf, name="qT_sb")       # 16 KiB/part
            kT_sb = resid.tile([P, H, L], bf, name="kT_sb")       # 16 KiB/part
            v_sb = resid.tile([P, L // P, F], bf, name="v_sb")    # 16 KiB/part
            # wo spans both phases
            wo_pool = ctx.enter_context(tc.tile_pool(name="wop", bufs=1))
            wo_res = wo_pool.tile([P, H, D], bf, name="wo_res")

            if "1" not in phases:
                # timing-attribution mode: phase 2 alone needs defined inputs
                nc.gpsimd.memset(qT_sb[:], 0.0)
                nc.gpsimd.memset(kT_sb[:], 0.0)
                nc.gpsimd.memset(v_sb[:], 0.0)
                nc.sync.dma_start(wo_res[:], wo[:])

            rep_ctx = ExitStack()
            if reps > 1:
                # timing mode: repeat the whole body in a hardware loop
                rep_ctx.enter_context(tc.For_i(0, reps, 1))

            # ============ phase 1: q,k then v projections ============
            with ExitStack() as p1:
                wres_pool = p1.enter_context(tc.tile_pool(name="wres", bufs=1))
                psA = p1.enter_context(tc.tile_pool(name="psA", bufs=4, space="PSUM"))

                wq_res = wres_pool.tile([P, H, KT, HD], bf, name="wq_res")
                wk_res = wres_pool.tile([P, H, KT, HD], bf, name="wk_res")
                wv_res = wres_pool.tile([P, KT, F], bf, name="wv_res")

                chunks = [(0, 256), (256, 256)] + [
                    (tb * TB, TB) for tb in range(1, NTB)
                ]
                if "1" in phases:
                    nc.sync.dma_start(wq_res[:, 0], wq[:, 0])
                else:
                    chunks = []
                for ci, (lo, ln) in enumerate(chunks):
                    nc.sync.dma_start(xr[:, :, lo : lo + ln], xT3[:, :, lo : lo + ln])
                    if ci == 0:
                        # remaining weight loads, finest-first for pipelining
                        nc.sync.dma_start(wk_res[:, 0], wk[:, 0])
                        for hh in range(1, H):
                            nc.sync.dma_start(wq_res[:, hh], wq[:, hh])
                            nc.sync.dma_start(wk_res[:, hh], wk[:, hh])
                        nc.sync.dma_start(wv_res[:], wv[:])
                        nc.sync.dma_start(wo_res[:], wo[:])
                    for wres, bias_pp, dst in (
                        (wq_res, bq_pp, qT_sb),
                        (wk_res, bk_pp, kT_sb),
                    ):
                        for h in range(H):
                            ps = psA.tile([P, TB], f32, tag="psA")
                            for kt in range(KT):
                                nc.tensor.matmul(
                                    ps[:, :ln],
                                    wres[:, h, kt],
                                    xr[:, kt, lo : lo + ln],
                                    start=(kt == 0),
                                    stop=(kt == KT - 1),
                                )
                            nc.scalar.activation(
                                dst[:, h, lo : lo + ln],
                                ps[:, :ln],
                                mybir.ActivationFunctionType.Identity,
                                bias=bias_pp[:, h : h + 1],
                                scale=1.0,
                            )

                # v projection for the whole sequence (x fully resident now).
                # bv is folded into bo on the host (softmax rows sum to 1).
                for q4 in range(L // P if "1" in phases else 0):
                    ps = psA.tile([P, F], f32, tag="psA")
                    for kt in range(KT):
                        nc.tensor.matmul(
                            ps[:],
                            xr[:, kt, q4 * P : (q4 + 1) * P],
                            wv_res[:, kt],
                            start=(kt == 0),
                            stop=(kt == KT - 1),
                        )
                    nc.vector.tensor_scalar_mul(v_sb[:, q4], ps[:], 1.0)

            # ============ phase 2: attention + o-projection ============
            with ExitStack() as p2:
                apool = p2.enter_context(tc.tile_pool(name="apool", bufs=1))
                ptpool = p2.enter_context(tc.tile_pool(name="ptpool", bufs=cfg["pts"]))
                spool = p2.enter_context(tc.tile_pool(name="spool", bufs=1))
                ostg = p2.enter_context(tc.tile_pool(name="ostg", bufs=2))
                psS = p2.enter_context(tc.tile_pool(name="psS", bufs=cfg["psS"], space="PSUM"))
                psPO = p2.enter_context(tc.tile_pool(name="psPO", bufs=cfg["psPO"], space="PSUM"))
                psR = p2.enter_context(tc.tile_pool(name="psR", bufs=cfg["psR"], space="PSUM"))
                psC = p2.enter_context(tc.tile_pool(name="psC", bufs=cfg["psC"], space="PSUM"))

                if "2" not in phases:
                    # keep the ExternalOutput written in phase-1-only builds
                    nc.sync.dma_start(o[:P, :TB], v_sb[:, 0])

                pts = {}
                po_h = {}
                rsum_h = {}
                att_tb = {}

                def col_off(tb, jt):
                    # columns left of the diagonal subtile are fully masked
                    jl = jt - 4 * tb
                    return jl * P if jl > 0 else 0

                def emit_score(tb, h, jt):
                    off = col_off(tb, jt)
                    w = TB - off
                    s = psS.tile([P, TB], f32, tag="s")
                    nc.tensor.matmul(
                        s[:, :w],
                        kT_sb[:, h, jt * P : (jt + 1) * P],
                        qT_sb[:, h, tb * TB + off : (tb + 1) * TB],
                        start=True,
                        stop=True,
                    )
                    if jt - 4 * tb >= 0:
                        # diagonal 128-col subtile: triangular mask
                        nc.vector.tensor_tensor(
                            s[:, :P], s[:, :P], tri[:], mybir.AluOpType.add
                        )
                    pt = ptpool.tile([P, TB], bf, tag="pt")
                    nc.scalar.activation(
                        pt[:, :w], s[:, :w], mybir.ActivationFunctionType.Exp
                    )
                    pts[(tb, h, jt)] = pt

                def emit_rp(tb, h, jt):
                    njt = 4 * (tb + 1)
                    off = col_off(tb, jt)
                    w = TB - off
                    pt = pts.pop((tb, h, jt))
                    if jt == 0:
                        po_h[(tb, h)] = psPO.tile([P, TB], f32, tag="po", name=f"po{tb}_{h}")
                        rsum_h[(tb, h)] = psR.tile([P, TB], f32, tag="rsum", name=f"rs{tb}_{h}")
                    nc.tensor.matmul(
                        rsum_h[(tb, h)][:, off:], ones_sq[:], pt[:, :w],
                        start=(jt == 0), stop=(jt == njt - 1),
                    )
                    nc.tensor.matmul(
                        po_h[(tb, h)][:, off:], v_sb[:, jt, h * HD : (h + 1) * HD], pt[:, :w],
                        start=(jt == 0), stop=(jt == njt - 1),
                    )
                    if jt == njt - 1:
                        emit_tail(tb, h)

                def emit_tail(tb, h):
                    po = po_h.pop((tb, h))
                    rsum = rsum_h.pop((tb, h))
                    # rsum is row-broadcast [P, TB]: reciprocal + normalize
                    # are plain elementwise DVE ops, no PE broadcast needed
                    bc = spool.tile([P, TB], f32, tag="bc")
                    nc.vector.reciprocal(bc[:], rsum[:])
                    nc.vector.tensor_tensor(
                        att_tb[tb][:, h, :], po[:], bc[:], mybir.AluOpType.mult
                    )

                oproj_queue = []
                ot_blk = {}
                o4 = o.rearrange("(tb tt p) d -> tb p tt d", p=P, tt=TB // P)

                def emit_oproj_group(tb, att_sb, ob, tt, last):
                    ps = psC.tile([P, TB], f32, tag="psC")
                    for h in range(H):
                        nc.tensor.matmul(
                            ps[:],
                            att_sb[:, h, tt * P : (tt + 1) * P],
                            wo_res[:, h, ob * TB : (ob + 1) * TB],
                            start=(h == 0),
                            stop=(h == H - 1),
                        )
                    ot = ot_blk[tb]
                    nc.vector.tensor_scalar_mul(
                        ot[:, tt, ob * TB : (ob + 1) * TB], ps[:], 1.0
                    )
                    if last:
                        # one store per token row-chunk, on the Pool queue so
                        # the sync queue only carries loads
                        nc.gpsimd.dma_start(o4[tb, :, tt], ot[:, tt])
                        if tt == TB // P - 1:
                            del ot_blk[tb]

                def emit_oproj(tb):
                    # queue the 16 groups; drained one per pipeline task so
                    # the PE-heavy o-projection meshes with the ACT-heavy
                    # exp stream of the next block's attention.  tt-outer so
                    # each 128-token row chunk completes (and stores) early.
                    att_sb = att_tb.pop(tb)
                    ot_blk[tb] = ostg.tile(
                        [P, TB // P, D], bf, tag="ostg", name=f"ot{tb}"
                    )
                    for tt in range(TB // P):
                        for ob in range(D // TB):
                            oproj_queue.append(
                                (tb, att_sb, ob, tt, ob == D // TB - 1)
                            )

                # one flat software pipeline across (tb, head, j-tile): the
                # score matmul leads the rsum/PV step by one so the PE never
                # waits on ACT's exp; the o-projection for block tb is
                # emitted right after its last head completes.
                tasks = []
                if "2" in phases:
                    for tb in range(NTB):
                        for h in range(H):
                            for jt in range(4 * (tb + 1)):
                                tasks.append((tb, h, jt))

                for tb in range(NTB):
                    att_tb[tb] = apool.tile([P, H, TB], bf, tag="att", name=f"att{tb}")

                STAG = cfg["stag"]
                if tasks:
                    for i in range(STAG):
                        emit_score(*tasks[i])
                    for i in range(STAG, len(tasks)):
                        emit_score(*tasks[i])
                        j = i - STAG
                        emit_rp(*tasks[j])
                        if tasks[j][0] != tasks[j + 1][0]:
                            # j was the last task of its block
                            emit_oproj(tasks[j][0])
                        if oproj_queue:
                            emit_oproj_group(*oproj_queue.pop(0))
                    for j in range(len(tasks) - STAG, len(tasks)):
                        emit_rp(*tasks[j])
                    emit_oproj(NTB - 1)
                    while oproj_queue:
                        emit_oproj_group(*oproj_queue.pop(0))

            rep_ctx.close()

    nc.compile()
    _CACHE[key] = nc
    return nc


def _in_maps(hidden_states, Wq, bq, Wk, bk, Wv, bv, Wo, bo):
    bfnp = mybir.dt.np(bf)
    hs = np.asarray(hidden_states, np.float32)
    Wq = np.asarray(Wq, np.float32)
    Wk = np.asarray(Wk, np.float32)
    Wv = np.asarray(Wv, np.float32)
    Wo = np.asarray(Wo, np.float32)
    bq = np.asarray(bq, np.float32)
    bk = np.asarray(bk, np.float32)
    bv = np.asarray(bv, np.float32)

    maps = []
    for b in range(B):
        xT = np.ascontiguousarray(hs[b].T).astype(bfnp)
        for g in range(G):
            sl = slice(g * F, (g + 1) * F)
            wqT = (Wq[sl, :].T * SCALE).astype(np.float32)   # (D, F)
            wkT = Wk[sl, :].T                                 # (D, F)
            wvT = Wv[sl, :].T                                 # (D, F)
            woT = Wo[:, sl].T                                 # (F, D)
            maps.append(
                {
                    "xT": xT,
                    "wq": np.ascontiguousarray(
                        wqT.reshape(KT, P, H, HD).transpose(1, 2, 0, 3)
                    ).astype(bfnp),
                    "wk": np.ascontiguousarray(
                        wkT.reshape(KT, P, H, HD).transpose(1, 2, 0, 3)
                    ).astype(bfnp),
                    "wv": np.ascontiguousarray(
                        wvT.reshape(KT, P, F).transpose(1, 0, 2)
                    ).astype(bfnp),
                    "wo": np.ascontiguousarray(
                        woT.reshape(H, HD, D).transpose(1, 0, 2)
                    ).astype(bfnp),
                    "bqv": np.ascontiguousarray(bq[sl] * SCALE),
                    "bkv": np.ascontiguousarray(bk[sl]),
                }
            )
    return maps


def kernel(hidden_states, Wq, bq, Wk, bk, Wv, bv, Wo, bo, **run_kwargs):
    nc = _build()
    maps = _in_maps(hidden_states, Wq, bq, Wk, bk, Wv, bv, Wo, bo)
    res = bass_utils.run_bass_kernel_spmd(
        nc, maps, core_ids=list(range(8)), **run_kwargs
    )
    # v-bias folded here: softmax rows sum to 1, so attn(v + bv) =
    # attn(v) + bv and the o-projection turns bv into a constant row.
    bo_eff = np.asarray(bo, np.float32) + np.asarray(Wo, np.float32) @ np.asarray(
        bv, np.float32
    )
    out = np.empty((B, L, D), np.float32)
    for b in range(B):
        acc = res.results[b * G]["o"].astype(np.float32)
        for g in range(1, G):
            acc = acc + res.results[b * G + g]["o"].astype(np.float32)
        out[b] = acc + bo_eff[None, :]
    _CACHE["last_res"] = res
    return out


# revision 27
# speedup vs baseline: 1.1331x; 1.1331x over previous
"""Causal multi-head attention (B=2, L=2048, D=2048, NH=16, HD=128) on 8
Trainium2 NeuronCores.

Sharding: core c = b*4 + g handles batch b and head-group g (4 heads).
Each core computes q/k/v projections for its 512 features, causal
attention for its 4 heads, and the partial o-projection
attn_out @ Wo[:, g_cols].T -> [L, D].  The host sums the 4 per-batch
partials and adds bo.

All matmul operands are bf16 (PSUM accumulation stays fp32): same PE
rate as full-rate f32r but half the HBM/SBUF traffic.  x^T is loaded
once into SBUF (64 KiB/partition) and reused for the q, k and v
projections; o partials are written back as bf16.

Structure:
  phase 1: q,k projections (wq+wk resident, x streamed in chunks into
           the resident x tile), then v projection for the whole
           sequence.  wo prefetched during this phase.
  phase 2: flash-style causal attention (scores in the transposed
           [k, q] layout; softmax without max-shift -- scores ~ N(0,1);
           row sums via ones-matmul; normalization by broadcast
           reciprocal) fused with the partial o-projection.
"""

import sys

for _p in ("/opt/trn_rl_repo",):
    if _p not in sys.path:
        sys.path.insert(0, _p)

import numpy as np
from contextlib import ExitStack

import concourse.bass as bass  # noqa: F401
import concourse.tile as tile
from concourse import bacc, mybir
from concourse import bass_utils

P = 128
B, L, D = 2, 2048, 2048
NH, HD = 16, 128
SCALE = HD ** -0.5
G = 8 // B            # head-groups per batch = 4
H = 4                 # heads per core
F = H * HD            # 512 features per core
TB = 512              # token block (q-block)
NTB = L // TB         # 4
KT = D // P           # 16 contraction tiles for projections

bf = mybir.dt.bfloat16
f32r = mybir.dt.float32r
f32 = mybir.dt.float32

_CACHE = {}


def _build(reps=1, phases="12", cfg=None):
    cfg = {**dict(stag=4, pts=5, psS=3, psPO=2, psR=1, psC=2), **(cfg or {})}
    key = ("nc", reps, phases, tuple(sorted(cfg.items())))
    if key in _CACHE:
        return _CACHE[key]

    nc = bacc.Bacc("TRN2", target_bir_lowering=False, debug=False, num_devices=8)

    xT = nc.dram_tensor("xT", [D, L], bf, kind="ExternalInput").ap()
    wq = nc.dram_tensor("wq", [P, H, KT, HD], bf, kind="ExternalInput").ap()
    wk = nc.dram_tensor("wk", [P, H, KT, HD], bf, kind="ExternalInput").ap()
    wv = nc.dram_tensor("wv", [P, KT, F], bf, kind="ExternalInput").ap()
    wo = nc.dram_tensor("wo", [P, H, D], bf, kind="ExternalInput").ap()
    bqv = nc.dram_tensor("bqv", [F], f32r, kind="ExternalInput").ap()
    bkv = nc.dram_tensor("bkv", [F], f32r, kind="ExternalInput").ap()
    o = nc.dram_tensor("o", [L, D], bf, kind="ExternalOutput").ap()

    xT3 = xT.rearrange("(kt p) t -> p kt t", p=P)

    with tile.TileContext(nc) as tc:
        with ExitStack() as ctx:
            ctx.enter_context(nc.allow_low_precision(reason="bf16 kernel"))
            consts = ctx.enter_context(tc.tile_pool(name="consts", bufs=1))
            resid = ctx.enter_context(tc.tile_pool(name="resid", bufs=1))

            # ---- constants ----
            # triangular additive mask [P, P] in [k, q] orientation:
            # keep (0.0) where k_local <= q_local, else -1e30
            tri = consts.tile([P, P], f32, name="tri")
            nc.gpsimd.memset(tri[:], 0.0)
            nc.gpsimd.affine_select(
                out=tri[:],
                in_=tri[:],
                compare_op=mybir.AluOpType.is_ge,
                fill=-1e30,
                base=0,
                pattern=[[1, P]],
                channel_multiplier=-1,
            )

            # all-ones stationary for the rsum colsum matmul; full 128x128
            # keeps the PE weight-load geometry uniform with every other
            # stationary, and makes the rsum output row-broadcast
            ones_sq = consts.tile([P, P], bf, name="ones_sq")
            nc.gpsimd.memset(ones_sq[:], 1.0)
            # per-partition bias tiles for the q/k copies: [P, H]
            bq_pp = consts.tile([P, H], f32, name="bq_pp")
            nc.scalar.dma_start(bq_pp[:], bqv.rearrange("(h p) -> p h", p=P).bitcast(f32))
            bk_pp = consts.tile([P, H], f32, name="bk_pp")
            nc.scalar.dma_start(bk_pp[:], bkv.rearrange("(h p) -> p h", p=P).bitcast(f32))

            # ---- persistent activations ----
            xr = resid.tile([P, KT, L], bf, name="xr")            # 64 KiB/part
            qT_sb = resid.tile([P, H, L], bf, name="qT_sb")       # 16 KiB/part
            kT_sb = resid.tile([P, H, L], bf, name="kT_sb")       # 16 KiB/part
            v_sb = resid.tile([P, L // P, F], bf, name="v_sb")    # 16 KiB/part
            # wo spans both phases
            wo_pool = ctx.enter_context(tc.tile_pool(name="wop", bufs=1))
            wo_res = wo_pool.tile([P, H, D], bf, name="wo_res")

            if "1" not in phases:
                # timing-attribution mode: phase 2 alone needs defined inputs
                nc.gpsimd.memset(qT_sb[:], 0.0)
                nc.gpsimd.memset(kT_sb[:], 0.0)
                nc.gpsimd.memset(v_sb[:], 0.0)
                nc.sync.dma_start(wo_res[:], wo[:])

            rep_ctx = ExitStack()
            if reps > 1:
                # timing mode: repeat the whole body in a hardware loop
                rep_ctx.enter_context(tc.For_i(0, reps, 1))

            # ============ phase 1: q,k then v projections ============
            with ExitStack() as p1:
                wres_pool = p1.enter_context(tc.tile_pool(name="wres", bufs=1))
                psA = p1.enter_context(tc.tile_pool(name="psA", bufs=4, space="PSUM"))

                wq_res = wres_pool.tile([P, H, KT, HD], bf, name="wq_res")
                wk_res = wres_pool.tile([P, H, KT, HD], bf, name="wk_res")
                wv_res = wres_pool.tile([P, KT, F], bf, name="wv_res")

                chunks = [(0, 256), (256, 256)] + [
                    (tb * TB, TB) for tb in range(1, NTB)
                ]
                if "1" in phases:
                    nc.sync.dma_start(wq_res[:, 0], wq[:, 0])
                else:
                    chunks = []
                for ci, (lo, ln) in enumerate(chunks):
                    nc.sync.dma_start(xr[:, :, lo : lo + ln], xT3[:, :, lo : lo + ln])
                    if ci == 0:
                        # remaining weight loads, finest-first for pipelining
                        nc.sync.dma_start(wk_res[:, 0], wk[:, 0])
                        for hh in range(1, H):
                            nc.sync.dma_start(wq_res[:, hh], wq[:, hh])
                            nc.sync.dma_start(wk_res[:, hh], wk[:, hh])
                        nc.sync.dma_start(wv_res[:], wv[:])
                        nc.sync.dma_start(wo_res[:], wo[:])
                    for wres, bias_pp, dst in (
                        (wq_res, bq_pp, qT_sb),
                        (wk_res, bk_pp, kT_sb),
                    ):
                        for h in range(H):
                            ps = psA.tile([P, TB], f32, tag="psA")
                            for kt in range(KT):
                                nc.tensor.matmul(
                                    ps[:, :ln],
                                    wres[:, h, kt],
                                    xr[:, kt, lo : lo + ln],
                                    start=(kt == 0),
                                    stop=(kt == KT - 1),
                                )
                            nc.scalar.activation(
                                dst[:, h, lo : lo + ln],
                                ps[:, :ln],
                                mybir.ActivationFunctionType.Identity,
                                bias=bias_pp[:, h : h + 1],
                                scale=1.0,
                            )

                # v projection for the whole sequence (x fully resident now).
                # bv is folded into bo on the host (softmax rows sum to 1).
                for q4 in range(L // P if "1" in phases else 0):
                    ps = psA.tile([P, F], f32, tag="psA")
                    for kt in range(KT):
                        nc.tensor.matmul(
                            ps[:],
                            xr[:, kt, q4 * P : (q4 + 1) * P],
                            wv_res[:, kt],
                            start=(kt == 0),
                            stop=(kt == KT - 1),
                        )
                    nc.vector.tensor_scalar_mul(v_sb[:, q4], ps[:], 1.0)

            # ============ phase 2: attention + o-projection ============
            with ExitStack() as p2:
                apool = p2.enter_context(tc.tile_pool(name="apool", bufs=1))
                ptpool = p2.enter_context(tc.tile_pool(name="ptpool", bufs=cfg["pts"]))
                spool = p2.enter_context(tc.tile_pool(name="spool", bufs=1))
                ostg = p2.enter_context(tc.tile_pool(name="ostg", bufs=2))
                psS = p2.enter_context(tc.tile_pool(name="psS", bufs=cfg["psS"], space="PSUM"))
                psPO = p2.enter_context(tc.tile_pool(name="psPO", bufs=cfg["psPO"], space="PSUM"))
                psR = p2.enter_context(tc.tile_pool(name="psR", bufs=cfg["psR"], space="PSUM"))
                psC = p2.enter_context(tc.tile_pool(name="psC", bufs=cfg["psC"], space="PSUM"))

                if "2" not in phases:
                    # keep the ExternalOutput written in phase-1-only builds
                    nc.sync.dma_start(o[:P, :TB], v_sb[:, 0])

                pts = {}
                po_h = {}
                rsum_h = {}
                att_tb = {}

                def col_off(tb, jt):
                    # columns left of the diagonal subtile are fully masked
                    jl = jt - 4 * tb
                    return jl * P if jl > 0 else 0

                def emit_score(tb, h, jt):
                    off = col_off(tb, jt)
                    w = TB - off
                    s = psS.tile([P, TB], f32, tag="s")
                    nc.tensor.matmul(
                        s[:, :w],
                        kT_sb[:, h, jt * P : (jt + 1) * P],
                        qT_sb[:, h, tb * TB + off : (tb + 1) * TB],
                        start=True,
                        stop=True,
                    )
                    if jt - 4 * tb >= 0:
                        # diagonal 128-col subtile: triangular mask
                        nc.vector.tensor_tensor(
                            s[:, :P], s[:, :P], tri[:], mybir.AluOpType.add
                        )
                    pt = ptpool.tile([P, TB], bf, tag="pt")
                    nc.scalar.activation(
                        pt[:, :w], s[:, :w], mybir.ActivationFunctionType.Exp
                    )
                    pts[(tb, h, jt)] = pt

                def emit_rp(tb, h, jt):
                    njt = 4 * (tb + 1)
                    off = col_off(tb, jt)
                    w = TB - off
                    pt = pts.pop((tb, h, jt))
                    if jt == 0:
                        po_h[(tb, h)] = psPO.tile([P, TB], f32, tag="po", name=f"po{tb}_{h}")
                        rsum_h[(tb, h)] = psR.tile([P, TB], f32, tag="rsum", name=f"rs{tb}_{h}")
                    nc.tensor.matmul(
                        rsum_h[(tb, h)][:, off:], ones_sq[:], pt[:, :w],
                        start=(jt == 0), stop=(jt == njt - 1),
                    )
                    nc.tensor.matmul(
                        po_h[(tb, h)][:, off:], v_sb[:, jt, h * HD : (h + 1) * HD], pt[:, :w],
                        start=(jt == 0), stop=(jt == njt - 1),
                    )
                    if jt == njt - 1:
                        emit_tail(tb, h)

                def emit_tail(tb, h):
                    po = po_h.pop((tb, h))
                    rsum = rsum_h.pop((tb, h))
                    # rsum is row-broadcast [P, TB]: reciprocal + normalize
                    # are plain elementwise DVE ops, no PE broadcast needed
                    bc = spool.tile([P, TB], f32, tag="bc")
                    nc.vector.reciprocal(bc[:], rsum[:])
                    nc.vector.tensor_tensor(
                        att_tb[tb][:, h, :], po[:], bc[:], mybir.AluOpType.mult
                    )

                oproj_queue = []
                ot_blk = {}
                o4 = o.rearrange("(tb tt p) d -> tb p tt d", p=P, tt=TB // P)

                def emit_oproj_group(tb, att_sb, ob, tt, last):
                    ps = psC.tile([P, TB], f32, tag="psC")
                    for h in range(H):
                        nc.tensor.matmul(
                            ps[:],
                            att_sb[:, h, tt * P : (tt + 1) * P],
                            wo_res[:, h, ob * TB : (ob + 1) * TB],
                            start=(h == 0),
                            stop=(h == H - 1),
                        )
                    ot = ot_blk[tb]
                    nc.vector.tensor_scalar_mul(
                        ot[:, tt, ob * TB : (ob + 1) * TB], ps[:], 1.0
                    )
                    if last:
                        # one store per token row-chunk, on the Pool queue so
                        # the sync queue only carries loads
                        nc.gpsimd.dma_start(o4[tb, :, tt], ot[:, tt])
                        if tt == TB // P - 1:
                            del ot_blk[tb]

                def emit_oproj(tb):
                    # queue the 16 groups; drained one per pipeline task so
                    # the PE-heavy o-projection meshes with the ACT-heavy
                    # exp stream of the next block's attention.  tt-outer so
                    # each 128-token row chunk completes (and stores) early.
                    att_sb = att_tb.pop(tb)
                    ot_blk[tb] = ostg.tile(
                        [P, TB // P, D], bf, tag="ostg", name=f"ot{tb}"
                    )
                    for tt in range(TB // P):
                        for ob in range(D // TB):
                            oproj_queue.append(
                                (tb, att_sb, ob, tt, ob == D // TB - 1)
                            )

                # one flat software pipeline across (tb, head, j-tile): the
                # score matmul leads the rsum/PV step by one so the PE never
                # waits on ACT's exp; the o-projection for block tb is
                # emitted right after its last head completes.
                tasks = []
                if "2" in phases:
                    for tb in range(NTB):
                        for h in range(H):
                            for jt in range(4 * (tb + 1)):
                                tasks.append((tb, h, jt))

                for tb in range(NTB):
                    att_tb[tb] = apool.tile([P, H, TB], bf, tag="att", name=f"att{tb}")

                STAG = cfg["stag"]
                if tasks:
                    for i in range(STAG):
                        emit_score(*tasks[i])
                    for i in range(STAG, len(tasks)):
                        emit_score(*tasks[i])
                        j = i - STAG
                        emit_rp(*tasks[j])
                        if tasks[j][0] != tasks[j + 1][0]:
                            # j was the last task of its block
                            emit_oproj(tasks[j][0])
                        if oproj_queue:
                            emit_oproj_group(*oproj_queue.pop(0))
                    for j in range(len(tasks) - STAG, len(tasks)):
                        emit_rp(*tasks[j])
                    emit_oproj(NTB - 1)
                    while oproj_queue:
                        emit_oproj_group(*oproj_queue.pop(0))

            rep_ctx.close()

    nc.compile()
    _CACHE[key] = nc
    return nc


def _in_maps(hidden_states, Wq, bq, Wk, bk, Wv, bv, Wo, bo):
    bfnp = mybir.dt.np(bf)
    hs = np.asarray(hidden_states, np.float32)
    Wq = np.asarray(Wq, np.float32)
    Wk = np.asarray(Wk, np.float32)
    Wv = np.asarray(Wv, np.float32)
    Wo = np.asarray(Wo, np.float32)
    bq = np.asarray(bq, np.float32)
    bk = np.asarray(bk, np.float32)
    bv = np.asarray(bv, np.float32)

    maps = []
    for b in range(B):
        xT = np.ascontiguousarray(hs[b].T).astype(bfnp)
        for g in range(G):
            sl = slice(g * F, (g + 1) * F)
            wqT = (Wq[sl, :].T * SCALE).astype(np.float32)   # (D, F)
            wkT = Wk[sl, :].T                                 # (D, F)
            wvT = Wv[sl, :].T                                 # (D, F)
            woT = Wo[:, sl].T                                 # (F, D)
            maps.append(
                {
                    "xT": xT,
                    "wq": np.ascontiguousarray(
                        wqT.reshape(KT, P, H, HD).transpose(1, 2, 0, 3)
                    ).astype(bfnp),
                    "wk": np.ascontiguousarray(
                        wkT.reshape(KT, P, H, HD).transpose(1, 2, 0, 3)
                    ).astype(bfnp),
                    "wv": np.ascontiguousarray(
                        wvT.reshape(KT, P, F).transpose(1, 0, 2)
                    ).astype(bfnp),
                    "wo": np.ascontiguousarray(
                        woT.reshape(H, HD, D).transpose(1, 0, 2)
                    ).astype(bfnp),
                    "bqv": np.ascontiguousarray(bq[sl] * SCALE),
                    "bkv": np.ascontiguousarray(bk[sl]),
                }
            )
    return maps


def kernel(hidden_states, Wq, bq, Wk, bk, Wv, bv, Wo, bo, **run_kwargs):
    nc = _build()
    maps = _in_maps(hidden_states, Wq, bq, Wk, bk, Wv, bv, Wo, bo)
    res = bass_utils.run_bass_kernel_spmd(
        nc, maps, core_ids=list(range(8)), **run_kwargs
    )
    # v-bias folded here: softmax rows sum to 1, so attn(v + bv) =
    # attn(v) + bv and the o-projection turns bv into a constant row.
    bo_eff = np.asarray(bo, np.float32) + np.asarray(Wo, np.float32) @ np.asarray(
        bv, np.float32
    )
    out = np.empty((B, L, D), np.float32)
    for b in range(B):
        acc = res.results[b * G]["o"].astype(np.float32)
        for g in range(1, G):
            acc = acc + res.results[b * G + g]["o"].astype(np.float32)
        out[b] = acc + bo_eff[None, :]
    _CACHE["last_res"] = res
    return out


# revision 28
# speedup vs baseline: 1.2149x; 1.0721x over previous
"""Causal multi-head attention (B=2, L=2048, D=2048, NH=16, HD=128) on 8
Trainium2 NeuronCores.

Sharding: core c = b*4 + g handles batch b and head-group g (4 heads).
Each core computes q/k/v projections for its 512 features, causal
attention for its 4 heads, and the partial o-projection
attn_out @ Wo[:, g_cols].T -> [L, D].  The host sums the 4 per-batch
partials and adds bo (bv is folded into bo host-side: softmax rows sum
to one, so attn(v + bv) = attn(v) + bv).

All matmul operands are bf16 (PSUM accumulation stays fp32): same PE
rate as full-rate f32r but half the HBM/SBUF traffic.

Single fused pipeline over 512-token blocks: the q/k/v projection
groups of block tb+1 are interleaved as fillers between the attention
tasks of block tb, so the ACT-bound exp stream and the DVE copies hide
behind the projection matmuls instead of binding in a separate
attention-only phase.  x^T streams through a 2-deep rotating SBUF
buffer (read once from HBM); o partials are written back as bf16 in
128-row chunks from a per-block staging tile.

Attention: scores in the transposed [k, q] layout; softmax without
max-shift (scores ~ N(0,1)); row sums via an all-ones 128x128
stationary matmul whose output is row-broadcast, making the
normalization a pair of plain elementwise DVE ops.
"""

import sys

for _p in ("/opt/trn_rl_repo",):
    if _p not in sys.path:
        sys.path.insert(0, _p)

import numpy as np
from contextlib import ExitStack

import concourse.bass as bass  # noqa: F401
import concourse.tile as tile
from concourse import bacc, mybir
from concourse import bass_utils

P = 128
B, L, D = 2, 2048, 2048
NH, HD = 16, 128
SCALE = HD ** -0.5
G = 8 // B            # head-groups per batch = 4
H = 4                 # heads per core
F = H * HD            # 512 features per core
TB = 512              # token block
NTB = L // TB         # 4
KT = D // P           # 16 contraction tiles for projections

bf = mybir.dt.bfloat16
f32r = mybir.dt.float32r
f32 = mybir.dt.float32

_CACHE = {}


def _build(reps=1, cfg=None):
    cfg = {**dict(stag=4, pts=5, psA=2, psS=3, psPO=2, psR=1), **(cfg or {})}
    key = ("nc", reps, tuple(sorted(cfg.items())))
    if key in _CACHE:
        return _CACHE[key]

    nc = bacc.Bacc("TRN2", target_bir_lowering=False, debug=False, num_devices=8)

    xT = nc.dram_tensor("xT", [D, L], bf, kind="ExternalInput").ap()
    wq = nc.dram_tensor("wq", [P, H, KT, HD], bf, kind="ExternalInput").ap()
    wk = nc.dram_tensor("wk", [P, H, KT, HD], bf, kind="ExternalInput").ap()
    wv = nc.dram_tensor("wv", [P, KT, F], bf, kind="ExternalInput").ap()
    wo = nc.dram_tensor("wo", [P, H, D], bf, kind="ExternalInput").ap()
    bqv = nc.dram_tensor("bqv", [F], f32r, kind="ExternalInput").ap()
    bkv = nc.dram_tensor("bkv", [F], f32r, kind="ExternalInput").ap()
    o = nc.dram_tensor("o", [L, D], bf, kind="ExternalOutput").ap()

    xT3 = xT.rearrange("(kt p) t -> p kt t", p=P)
    o4 = o.rearrange("(tb tt p) d -> tb p tt d", p=P, tt=TB // P)

    with tile.TileContext(nc) as tc:
        with ExitStack() as ctx:
            ctx.enter_context(nc.allow_low_precision(reason="bf16 kernel"))
            consts = ctx.enter_context(tc.tile_pool(name="consts", bufs=1))
            resid = ctx.enter_context(tc.tile_pool(name="resid", bufs=1))
            wpool = ctx.enter_context(tc.tile_pool(name="wpool", bufs=1))

            # ---- constants ----
            # triangular additive mask [P, P] in [k, q] orientation:
            # keep (0.0) where k_local <= q_local, else -1e30
            tri = consts.tile([P, P], f32, name="tri")
            nc.gpsimd.memset(tri[:], 0.0)
            nc.gpsimd.affine_select(
                out=tri[:],
                in_=tri[:],
                compare_op=mybir.AluOpType.is_ge,
                fill=-1e30,
                base=0,
                pattern=[[1, P]],
                channel_multiplier=-1,
            )
            # all-ones stationary for the rsum colsum matmul; full 128x128
            # keeps the PE weight-load geometry uniform with every other
            # stationary, and makes the rsum output row-broadcast
            ones_sq = consts.tile([P, P], bf, name="ones_sq")
            nc.gpsimd.memset(ones_sq[:], 1.0)
            # per-partition bias tiles for the q/k copies: [P, H]
            bq_pp = consts.tile([P, H], f32, name="bq_pp")
            nc.scalar.dma_start(bq_pp[:], bqv.rearrange("(h p) -> p h", p=P).bitcast(f32))
            bk_pp = consts.tile([P, H], f32, name="bk_pp")
            nc.scalar.dma_start(bk_pp[:], bkv.rearrange("(h p) -> p h", p=P).bitcast(f32))

            # ---- persistent activations / weights ----
            qT_sb = resid.tile([P, H, L], bf, name="qT_sb")       # 16 KiB/part
            kT_sb = resid.tile([P, H, L], bf, name="kT_sb")       # 16 KiB/part
            v_sb = resid.tile([P, L // P, F], bf, name="v_sb")    # 16 KiB/part
            wq_res = wpool.tile([P, H, KT, HD], bf, name="wq_res")
            wk_res = wpool.tile([P, H, KT, HD], bf, name="wk_res")
            wv_res = wpool.tile([P, KT, F], bf, name="wv_res")
            wo_res = wpool.tile([P, H, D], bf, name="wo_res")

            rep_ctx = ExitStack()
            if reps > 1:
                # timing mode: repeat the whole body in a hardware loop
                rep_ctx.enter_context(tc.For_i(0, reps, 1))

            with ExitStack() as ph:
                xpool = ph.enter_context(tc.tile_pool(name="xpool", bufs=2))
                apool = ph.enter_context(tc.tile_pool(name="apool", bufs=1))
                ptpool = ph.enter_context(
                    tc.tile_pool(name="ptpool", bufs=cfg["pts"])
                )
                spool = ph.enter_context(tc.tile_pool(name="spool", bufs=1))
                ostg = ph.enter_context(tc.tile_pool(name="ostg", bufs=1))
                psA = ph.enter_context(
                    tc.tile_pool(name="psA", bufs=cfg["psA"], space="PSUM")
                )
                psS = ph.enter_context(
                    tc.tile_pool(name="psS", bufs=cfg["psS"], space="PSUM")
                )
                psPO = ph.enter_context(
                    tc.tile_pool(name="psPO", bufs=cfg["psPO"], space="PSUM")
                )
                psR = ph.enter_context(
                    tc.tile_pool(name="psR", bufs=cfg["psR"], space="PSUM")
                )

                # ---------- projection work items ----------
                xt = {}

                def dma_chunk(tb):
                    xt[tb] = xpool.tile([P, KT, TB], bf, tag="xt", name=f"xt{tb}")
                    lo = tb * TB
                    if tb == 0:
                        # split for an early start on the first groups
                        nc.sync.dma_start(xt[0][:, :, :256], xT3[:, :, :256])
                    else:
                        nc.sync.dma_start(xt[tb][:], xT3[:, :, lo : lo + TB])

                def emit_qk_group(tb, wres, bias_pp, dst, h, lo=0, ln=TB):
                    ps = psA.tile([P, TB], f32, tag="psA")
                    for kt in range(KT):
                        nc.tensor.matmul(
                            ps[:, :ln],
                            wres[:, h, kt],
                            xt[tb][:, kt, lo : lo + ln],
                            start=(kt == 0),
                            stop=(kt == KT - 1),
                        )
                    nc.scalar.activation(
                        dst[:, h, tb * TB + lo : tb * TB + lo + ln],
                        ps[:, :ln],
                        mybir.ActivationFunctionType.Identity,
                        bias=bias_pp[:, h : h + 1],
                        scale=1.0,
                    )

                def emit_v_quarter(q4):
                    tb, qq = q4 // (TB // P), q4 % (TB // P)
                    ps = psA.tile([P, F], f32, tag="psA")
                    for kt in range(KT):
                        nc.tensor.matmul(
                            ps[:],
                            xt[tb][:, kt, qq * P : (qq + 1) * P],
                            wv_res[:, kt],
                            start=(kt == 0),
                            stop=(kt == KT - 1),
                        )
                    nc.vector.tensor_scalar_mul(v_sb[:, q4], ps[:], 1.0)

                def proj_items(tb):
                    items = []
                    for wres, bias_pp, dst in (
                        (wq_res, bq_pp, qT_sb),
                        (wk_res, bk_pp, kT_sb),
                    ):
                        for h in range(H):
                            items.append(
                                lambda tb=tb, wres=wres, b=bias_pp, d=dst, h=h:
                                emit_qk_group(tb, wres, b, d, h)
                            )
                    for q4 in range(tb * (TB // P), (tb + 1) * (TB // P)):
                        items.append(lambda q4=q4: emit_v_quarter(q4))
                    return items

                # ---------- attention work items ----------
                pts = {}
                po_h = {}
                rsum_h = {}
                att_tb = {}
                ot_blk = {}

                def col_off(tb, jt):
                    # columns left of the diagonal subtile are fully masked
                    jl = jt - 4 * tb
                    return jl * P if jl > 0 else 0

                def emit_score(tb, h, jt):
                    off = col_off(tb, jt)
                    w = TB - off
                    s = psS.tile([P, TB], f32, tag="s")
                    nc.tensor.matmul(
                        s[:, :w],
                        kT_sb[:, h, jt * P : (jt + 1) * P],
                        qT_sb[:, h, tb * TB + off : (tb + 1) * TB],
                        start=True,
                        stop=True,
                    )
                    if jt - 4 * tb >= 0:
                        # diagonal 128-col subtile: triangular mask
                        nc.vector.tensor_tensor(
                            s[:, :P], s[:, :P], tri[:], mybir.AluOpType.add
                        )
                    pt = ptpool.tile([P, TB], bf, tag="pt")
                    nc.scalar.activation(
                        pt[:, :w], s[:, :w], mybir.ActivationFunctionType.Exp
                    )
                    pts[(tb, h, jt)] = pt

                def emit_rp(tb, h, jt):
                    njt = 4 * (tb + 1)
                    off = col_off(tb, jt)
                    w = TB - off
                    pt = pts.pop((tb, h, jt))
                    if jt == 0:
                        po_h[(tb, h)] = psPO.tile(
                            [P, TB], f32, tag="po", name=f"po{tb}_{h}"
                        )
                        rsum_h[(tb, h)] = psR.tile(
                            [P, TB], f32, tag="rsum", name=f"rs{tb}_{h}"
                        )
                    nc.tensor.matmul(
                        rsum_h[(tb, h)][:, off:], ones_sq[:], pt[:, :w],
                        start=(jt == 0), stop=(jt == njt - 1),
                    )
                    nc.tensor.matmul(
                        po_h[(tb, h)][:, off:],
                        v_sb[:, jt, h * HD : (h + 1) * HD],
                        pt[:, :w],
                        start=(jt == 0), stop=(jt == njt - 1),
                    )
                    if jt == njt - 1:
                        emit_tail(tb, h)

                def emit_tail(tb, h):
                    po = po_h.pop((tb, h))
                    rsum = rsum_h.pop((tb, h))
                    # rsum is row-broadcast [P, TB]: reciprocal + normalize
                    # are plain elementwise DVE ops, no PE broadcast needed
                    bc = spool.tile([P, TB], f32, tag="bc")
                    nc.vector.reciprocal(bc[:], rsum[:])
                    nc.vector.tensor_tensor(
                        att_tb[tb][:, h, :], po[:], bc[:], mybir.AluOpType.mult
                    )

                oproj_queue = []

                def emit_oproj_group(tb, att_sb, ob, tt, last):
                    ps = psA.tile([P, TB], f32, tag="psA")
                    for h in range(H):
                        nc.tensor.matmul(
                            ps[:],
                            att_sb[:, h, tt * P : (tt + 1) * P],
                            wo_res[:, h, ob * TB : (ob + 1) * TB],
                            start=(h == 0),
                            stop=(h == H - 1),
                        )
                    ot = ot_blk[tb]
                    nc.vector.tensor_scalar_mul(
                        ot[:, tt, ob * TB : (ob + 1) * TB], ps[:], 1.0
                    )
                    if last:
                        # one store per 128-token row chunk, on the Pool
                        # queue so the sync queue only carries loads
                        nc.gpsimd.dma_start(o4[tb, :, tt], ot[:, tt])
                        if tt == TB // P - 1:
                            del ot_blk[tb]

                def emit_oproj(tb):
                    # queue the 16 groups; drained one per pipeline task.
                    # tt-outer so each row chunk completes (and stores) early
                    att_sb = att_tb.pop(tb)
                    ot_blk[tb] = ostg.tile(
                        [P, TB // P, D], bf, tag="ostg", name=f"ot{tb}"
                    )
                    for tt in range(TB // P):
                        for ob in range(D // TB):
                            oproj_queue.append(
                                (tb, att_sb, ob, tt, ob == D // TB - 1)
                            )

                # ---------- the fused schedule ----------
                nc.sync.dma_start(wq_res[:, 0], wq[:, 0])
                dma_chunk(0)
                nc.sync.dma_start(wk_res[:, 0], wk[:, 0])
                for hh in range(1, H):
                    nc.sync.dma_start(wq_res[:, hh], wq[:, hh])
                    nc.sync.dma_start(wk_res[:, hh], wk[:, hh])
                nc.sync.dma_start(xt[0][:, :, 256:], xT3[:, :, 256:512])
                nc.sync.dma_start(wv_res[:], wv[:])
                nc.sync.dma_start(wo_res[:], wo[:])

                # block-0 projections up front (nothing to interleave with);
                # 256-wide halves so compute starts on the first x half
                for lo in (0, 256):
                    for wres, bias_pp, dst in (
                        (wq_res, bq_pp, qT_sb),
                        (wk_res, bk_pp, kT_sb),
                    ):
                        for h in range(H):
                            emit_qk_group(0, wres, bias_pp, dst, h, lo=lo, ln=256)
                dma_chunk(1)
                for q4 in range(TB // P):
                    emit_v_quarter(q4)

                # flat task list; fillers (projections of block tb+1, x-chunk
                # DMA of block tb+2) attached to the rp index where they run
                tasks = []
                fillers = {}
                for tb in range(NTB):
                    start = len(tasks)
                    for h in range(H):
                        for jt in range(4 * (tb + 1)):
                            tasks.append((tb, h, jt))
                    n = len(tasks) - start
                    if tb + 1 < NTB:
                        items = proj_items(tb + 1)
                        if tb + 2 < NTB:
                            items.insert(0, lambda tb=tb: dma_chunk(tb + 2))
                        for idx, it in enumerate(items):
                            pos = start + (idx * n) // len(items)
                            fillers.setdefault(pos, []).append(it)

                for tb in range(NTB):
                    att_tb[tb] = apool.tile(
                        [P, H, TB], bf, tag="att", name=f"att{tb}"
                    )

                STAG = cfg["stag"]
                for i in range(STAG):
                    emit_score(*tasks[i])
                for i in range(STAG, len(tasks) + STAG):
                    if i < len(tasks):
                        emit_score(*tasks[i])
                    j = i - STAG
                    emit_rp(*tasks[j])
                    for it in fillers.get(j, ()):
                        it()
                    if j + 1 == len(tasks) or tasks[j][0] != tasks[j + 1][0]:
                        # j was the last task of its block
                        emit_oproj(tasks[j][0])
                    if oproj_queue:
                        emit_oproj_group(*oproj_queue.pop(0))
                while oproj_queue:
                    emit_oproj_group(*oproj_queue.pop(0))

            rep_ctx.close()

    nc.compile()
    _CACHE[key] = nc
    return nc


def _in_maps(hidden_states, Wq, bq, Wk, bk, Wv, bv, Wo, bo):
    bfnp = mybir.dt.np(bf)
    hs = np.asarray(hidden_states, np.float32)
    Wq = np.asarray(Wq, np.float32)
    Wk = np.asarray(Wk, np.float32)
    Wv = np.asarray(Wv, np.float32)
    Wo = np.asarray(Wo, np.float32)
    bq = np.asarray(bq, np.float32)
    bk = np.asarray(bk, np.float32)

    maps = []
    for b in range(B):
        xT = np.ascontiguousarray(hs[b].T).astype(bfnp)
        for g in range(G):
            sl = slice(g * F, (g + 1) * F)
            wqT = (Wq[sl, :].T * SCALE).astype(np.float32)   # (D, F)
            wkT = Wk[sl, :].T                                 # (D, F)
            wvT = Wv[sl, :].T                                 # (D, F)
            woT = Wo[:, sl].T                                 # (F, D)
            maps.append(
                {
                    "xT": xT,
                    "wq": np.ascontiguousarray(
                        wqT.reshape(KT, P, H, HD).transpose(1, 2, 0, 3)
                    ).astype(bfnp),
                    "wk": np.ascontiguousarray(
                        wkT.reshape(KT, P, H, HD).transpose(1, 2, 0, 3)
                    ).astype(bfnp),
                    "wv": np.ascontiguousarray(
                        wvT.reshape(KT, P, F).transpose(1, 0, 2)
                    ).astype(bfnp),
                    "wo": np.ascontiguousarray(
                        woT.reshape(H, HD, D).transpose(1, 0, 2)
                    ).astype(bfnp),
                    "bqv": np.ascontiguousarray(bq[sl] * SCALE),
                    "bkv": np.ascontiguousarray(bk[sl]),
                }
            )
    return maps


def kernel(hidden_states, Wq, bq, Wk, bk, Wv, bv, Wo, bo, **run_kwargs):
    nc = _build()
    maps = _in_maps(hidden_states, Wq, bq, Wk, bk, Wv, bv, Wo, bo)
    res = bass_utils.run_bass_kernel_spmd(
        nc, maps, core_ids=list(range(8)), **run_kwargs
    )
    # v-bias folded here: softmax rows sum to 1, so attn(v + bv) =
    # attn(v) + bv and the o-projection turns bv into a constant row.
    bo_eff = np.asarray(bo, np.float32) + np.asarray(Wo, np.float32) @ np.asarray(
        bv, np.float32
    )
    out = np.empty((B, L, D), np.float32)
    for b in range(B):
        acc = res.results[b * G]["o"].astype(np.float32)
        for g in range(1, G):
            acc = acc + res.results[b * G + g]["o"].astype(np.float32)
        out[b] = acc + bo_eff[None, :]
    _CACHE["last_res"] = res
    return out
